# revision 57
# baseline (speedup 1.0000x reference)
"""WaveNet-style gated dilated conv layer on 8 Trainium2 NeuronCores.

Strategy: data-parallel over batch (B=8 -> 1 batch element per core).
Per core (batch b):
  z_tan = sum_k Wt[k] @ x[:, t-d*(2-k)] + Wct @ cond          (bf16 matmuls)
  z_sig = same for the sigmoid half, but in fp8e4m3 with DoubleRow
          perf mode: two conv taps (or tap2+cond) are packed into the two
          DoubleRow k-planes, halving PE time for this half.  The sigmoid
          gate's derivative is <= 1/4, so the fp8 quantization error on
          z_sig stays within the 2e-2 gate (measured ~1.7e-2 end to end).
  h   = tanh(z_tan) * sigmoid(z_sig)     (ACT activations + DVE multiply)
  out = W_out @ h, skip = W_skip @ h     (bf16 1x1 convs)
Outputs are staged and DMA'd as bf16 (halves HBM write traffic) and
converted to fp32 on host.  PSUM->SBUF conversion copies are split
across DVE (out) and GpSimd/Pool (skip); the fp8 copy of cond is
generated on-chip (alternating DVE/Pool) from the bf16 cond tile.

Engine-budget summary per core (target ~60us wall):
  PE ~56us, ACT ~40us, DVE ~43us, Pool ~42us, DMA ~57us.

TRN2 matmul instructions only have room for a single semaphore wait, so
input DMAs are "observed" by the PE via standalone ldweights before the
first matmul that would otherwise combine a DMA wait with a PSUM WAR
wait, and the out/skip matmuls are ordered so the DVE/Pool WAR waits
are each covered by a single semaphore.
"""

import sys

for _p in ("/opt/trn_rl_repo",):
    if _p not in sys.path:
        sys.path.append(_p)

from contextlib import ExitStack

import ml_dtypes
import numpy as np

import bass_rust
import concourse.bacc as bacc
import concourse.bass as bass
import concourse.tile as tile
from concourse import mybir
from concourse.bass_utils import run_bass_kernel_spmd

B, CIN, T = 8, 128, 16384
R, S, CC, KW = 128, 128, 80, 3
NT = 512           # time-tile width (one PSUM bank of fp32)
DRN = 256          # DoubleRow matmul moving width (rhs free = 2*DRN = 512)
N_CORES = 8

BF16 = mybir.dt.bfloat16
FP32 = mybir.dt.float32
F8 = mybir.dt.float8e4
U8 = mybir.dt.uint8
AF = mybir.ActivationFunctionType
DR = mybir.MatmulPerfMode.DoubleRow

_built = {}
_TRACE = False        # set True (e.g. by a test harness) to capture an NTFF profile
_last_results = None  # BassKernelResults of the most recent run


# Streaming chunk widths: small at the head (fast first-compute), large in
# the middle (few DMA triggers), small at the tail (fast final drain).
CHUNK_WIDTHS = [512, 1536] + [2560] * 5 + [512, 512, 512]
assert sum(CHUNK_WIDTHS) == T
PREFETCH = 2         # chunk lookahead beyond the current group

# Weights blob layout (bytes per partition, all fields 4B-aligned).
# One DMA loads every weight; views are carved out with slice+bitcast.
_WT_OFF = 0                     # [128, 3*128] bf16  (tanh conv taps, lhsT)
_WOS_OFF = _WT_OFF + 3 * 128 * 2   # [128, 256] bf16 (out|skip lhsT)
_ZB_OFF = _WOS_OFF + 256 * 2       # [128, 2] fp32   (z biases)
_WS1_OFF = _ZB_OFF + 2 * 4         # [128, 2, 128] f8 (sig DR taps 0,1)
_WS2_OFF = _WS1_OFF + 256          # [128, 2, 128] f8 (sig DR tap2 | cond)
_WCT_OFF = _WS2_OFF + 256          # [80, 128] bf16  (tanh cond lhsT)
_WBLOB = _WCT_OFF + 128 * 2


def _pair_ap(t_ap, offset, stride, n):
    """AP reading two n-wide planes at (offset, offset+stride): [128, 2, n]."""
    ap = t_ap.copy()
    ap.ap = bass_rust.VecI64Pair([ap.ap[0], [stride, 2], [1, n]])
    ap.offset = offset
    return ap


def _build(dilation: int, t_total: int = T, chunk_widths=None) -> bass.Bass:
    d = dilation
    pad = d * (KW - 1)
    if chunk_widths is None:
        chunk_widths = CHUNK_WIDTHS if t_total == T else [512] * (t_total // 512)
    assert sum(chunk_widths) == t_total
    chunk_starts = [sum(chunk_widths[:i]) for i in range(len(chunk_widths))]
    nch = len(chunk_widths)

    nc = bacc.Bacc("TRN2", target_bir_lowering=False, debug=False, num_devices=N_CORES)

    xbf = nc.declare_dram_parameter("xbf", [CIN, pad + t_total], BF16, isOutput=False)
    # fp8 planes: [0] = x (causal-padded), [1] = cond (padded to 128 rows and
    # pad leading zeros so both planes slice identically per chunk)
    xc8 = nc.declare_dram_parameter("xc8", [CIN, 2, pad + t_total], F8, isOutput=False)
    condbf = nc.declare_dram_parameter("condbf", [CC, t_total], BF16, isOutput=False)
    wblob = nc.declare_dram_parameter("wblob", [128, _WBLOB], U8, isOutput=False)

    out = nc.declare_dram_parameter("out", [R, t_total], BF16, isOutput=True)
    skip = nc.declare_dram_parameter("skip", [S, t_total], BF16, isOutput=True)

    with tile.TileContext(nc) as tc, ExitStack() as ctx:
        consts = ctx.enter_context(tc.tile_pool(name="consts", bufs=1))
        inpool = ctx.enter_context(tc.tile_pool(name="inp", bufs=PREFETCH + 1))
        hpool = ctx.enter_context(tc.tile_pool(name="h", bufs=3))
        opool = ctx.enter_context(tc.tile_pool(name="o", bufs=3))
        zpsum = ctx.enter_context(tc.tile_pool(name="zpsum", bufs=2, space="PSUM"))
        opsum = ctx.enter_context(tc.tile_pool(name="opsum", bufs=2, space="PSUM"))

        xbf_tiles = [None] * nch
        c8_tiles = [None] * nch   # combined fp8 tile: [x8 pad+W | c8 W]
        cbf_tiles = [None] * nch

        def load_chunk(g):
            gs, gw = chunk_starts[g], chunk_widths[g]
            xt = inpool.tile([CIN, pad + gw], BF16, tag="xbf")
            nc.sync.dma_start(xt[:], xbf[:, gs : gs + pad + gw])
            ct = inpool.tile([CC, gw], BF16, tag="cbf")
            nc.sync.dma_start(ct[:], condbf[:, gs : gs + gw])
            c8 = inpool.tile([CIN, 2, pad + gw], F8, tag="xc8")
            # two plain 2D transfers: a single 3D (two-segment-per-partition)
            # descriptor pattern wedges the HWDGE ring
            nc.sync.dma_start(c8[:, 0], xc8[:, 0, gs : gs + pad + gw])
            nc.sync.dma_start(c8[:, 1], xc8[:, 1, gs : gs + pad + gw])
            xbf_tiles[g], cbf_tiles[g], c8_tiles[g] = xt, ct, c8

        load_chunk(0)
        wsb = consts.tile([128, _WBLOB], U8)
        nc.sync.dma_start(wsb[:], wblob[:])
        load_chunk(1)

        # weight views
        wt_sb = wsb[:, _WT_OFF : _WT_OFF + 3 * 128 * 2].bitcast(BF16)
        wos_sb = wsb[:, _WOS_OFF : _WOS_OFF + 512].bitcast(BF16)
        zb_sb = wsb[:, _ZB_OFF : _ZB_OFF + 8].bitcast(FP32)
        ws1_sb = wsb[:, _WS1_OFF : _WS1_OFF + 256].bitcast(F8)
        ws2_sb = wsb[:, _WS2_OFF : _WS2_OFF + 256].bitcast(F8)
        wct_sb = wsb[0:CC, _WCT_OFF : _WCT_OFF + 256].bitcast(BF16)
        ws1_ap = _pair_ap(ws1_sb, ws1_sb.offset, 128, 128)
        ws2_ap = _pair_ap(ws2_sb, ws2_sb.offset, 128, 128)

        # Warm-up during the input-load head: matmuls on uninitialized SBUF
        # kick the PE HAM to 8/8 before real work arrives, and two 1-column
        # activations trigger the tanh/sigmoid table load.
        garbage = consts.tile([CIN, 128], BF16)
        act_sink = consts.tile([R, 1], FP32)
        nc.vector.memset(garbage[:], 0.0)
        nc.vector.memset(act_sink[:], 0.0)
        for _ in range(12):
            wz = opsum.tile([R, 128], FP32, tag="ops")
            nc.tensor.matmul(wz[:], garbage[:, 0:R], garbage[:], start=True, stop=True)
        nc.scalar.activation(act_sink[:], act_sink[:], AF.Tanh, bias=zb_sb[:, 0:1])
        nc.scalar.activation(act_sink[:], act_sink[:], AF.Sigmoid, bias=zb_sb[:, 1:2])

        # Output DMA triggers are deferred by one chunk group so their waits
        # (on the staging copies) are satisfied when the SP sequencer reaches
        # them — an unsatisfied wait would head-of-line block the input
        # triggers queued behind them on the same HWDGE ring.
        pending_out = []

        def flush_out():
            while pending_out:
                gs, gw, oss = pending_out.pop()
                nc.sync.dma_start(out[:, gs : gs + gw], oss[:, 0:gw])
                nc.sync.dma_start(skip[:, gs : gs + gw], oss[:, gw : 2 * gw])

        for g in range(nch):
            gs, gw = chunk_starts[g], chunk_widths[g]
            for gg in range(g + 1, min(g + PREFETCH + 1, nch)):
                if xbf_tiles[gg] is None:
                    load_chunk(gg)
            xt, ct, c8 = xbf_tiles[g], cbf_tiles[g], c8_tiles[g]
            c8ap = c8[:]
            # let PE observe the chunk DMA sems on standalone ldweights
            # so no accumulating matmul needs two waits
            nc.tensor.ldweights(xt[:, 0:R])
            nc.tensor.ldweights(ct[:, 0:R])
            nc.tensor.ldweights(c8[:, 0, 0:R])

            oss = opool.tile([R, 2 * gw], BF16, tag="oss")  # [out gw | skip gw]
            for l0 in range(0, gw, NT):
                w = min(NT, gw - l0)
                ztan = zpsum.tile([R, w], FP32, tag="ztan")
                zsig = zpsum.tile([R, w], FP32, tag="zsig")
                # tanh half: plain bf16 matmuls
                for k in range(KW):
                    nc.tensor.matmul(
                        ztan[:], wt_sb[:, k * R : (k + 1) * R],
                        xt[:, l0 + d * k : l0 + d * k + w],
                        start=(k == 0), stop=False,
                    )
                nc.tensor.matmul(
                    ztan[:], wct_sb, ct[0:CC, l0 : l0 + w],
                    start=False, stop=True,
                )
                # sigmoid half: fp8 DoubleRow, two k-planes per matmul
                for o in range(0, w, DRN):
                    n = min(DRN, w - o)
                    rhs1 = _pair_ap(c8ap, l0 + o, d, n)
                    nc.tensor.matmul(
                        zsig[:, o : o + n], ws1_ap, rhs1,
                        start=True, stop=False, perf_mode=DR,
                    )
                    rhs2 = _pair_ap(c8ap, l0 + o + pad, pad + gw, n)
                    nc.tensor.matmul(
                        zsig[:, o : o + n], ws2_ap, rhs2,
                        start=False, stop=True, perf_mode=DR,
                    )

                th = hpool.tile([R, w], BF16, tag="th")
                nc.scalar.activation(th[:], ztan[:], AF.Tanh, bias=zb_sb[:, 0:1])
                sg = hpool.tile([R, w], BF16, tag="sg")
                nc.scalar.activation(sg[:], zsig[:], AF.Sigmoid, bias=zb_sb[:, 1:2])
                h = hpool.tile([R, w], BF16, tag="h")
                nc.vector.tensor_mul(h[:], th[:], sg[:])

                # out and skip land in the two bank-aligned halves of one
                # PSUM tile; a single DVE copy converts both to bf16, writing
                # the out/skip staging regions via a two-plane strided AP.
                # out-mm's DVE wait (on h) also covers the copy WAR; skip-mm
                # then needs no wait at all (PE program order).
                ops = opsum.tile([R, 2 * NT], FP32, tag="ops")
                nc.tensor.matmul(ops[:, 0:NT], wos_sb[:, 0:R], h[:], start=True, stop=True)
                nc.tensor.matmul(
                    ops[:, NT : 2 * NT], wos_sb[:, R : R + S], h[:], start=True, stop=True
                )
                nc.vector.tensor_copy(
                    _pair_ap(oss[:], l0, gw, w), _pair_ap(ops[:], 0, NT, w)
                )

            flush_out()
            pending_out.append((gs, gw, oss))
            if g == nch - 1:
                flush_out()

    nc.compile()
    return nc


def _pack_inputs(x, cond, w_conv, b_conv, w_cond, b_cond, w_out, w_skip, pad,
                 t_total=T, b_total=B):
    bf = ml_dtypes.bfloat16
    f8 = ml_dtypes.float8_e4m3

    xbf = np.zeros((b_total, CIN, pad + t_total), dtype=bf)
    xbf[:, :, pad:] = x.astype(bf)
    xc8 = np.zeros((b_total, CIN, 2, pad + t_total), dtype=f8)
    xc8[:, :, 0, pad:] = x.astype(f8)
    xc8[:, 0:CC, 1, pad:] = cond.astype(f8)
    cbf = np.ascontiguousarray(cond.astype(bf))

    blob = np.zeros((128, _WBLOB), dtype=np.uint8)

    def put(off, a):
        v = np.ascontiguousarray(a).view(np.uint8)
        blob[: v.shape[0], off : off + v.shape[1]] = v

    wt = np.concatenate([w_conv[0:R, :, k].T for k in range(KW)], axis=1).astype(bf)
    put(_WT_OFF, wt)
    wos = np.concatenate([w_out[:, :, 0].T, w_skip[:, :, 0].T], axis=1).astype(bf)
    put(_WOS_OFF, wos)
    zbias = np.stack(
        [b_conv[:R] + b_cond[:R], b_conv[R:] + b_cond[R:]], axis=1
    ).astype(np.float32)
    put(_ZB_OFF, zbias)
    ws1 = np.concatenate(
        [w_conv[R:, :, 0].T.astype(f8), w_conv[R:, :, 1].T.astype(f8)], axis=1
    )
    put(_WS1_OFF, ws1)
    wc8 = np.zeros((CIN, R), dtype=f8)
    wc8[0:CC] = w_cond[R:, :, 0].T.astype(f8)
    ws2 = np.concatenate([w_conv[R:, :, 2].T.astype(f8), wc8], axis=1)
    put(_WS2_OFF, ws2)
    wct = np.zeros((128, R), dtype=bf)
    wct[0:CC] = w_cond[0:R, :, 0].T.astype(bf)
    put(_WCT_OFF, wct)

    return xbf, xc8, cbf, blob


def kernel(**inputs):
    x = np.asarray(inputs["x"], dtype=np.float32)
    cond = np.asarray(inputs["cond"], dtype=np.float32)
    w_conv = np.asarray(inputs["w_conv"], dtype=np.float32)
    b_conv = np.asarray(inputs["b_conv"], dtype=np.float32)
    w_cond = np.asarray(inputs["w_cond"], dtype=np.float32)
    b_cond = np.asarray(inputs["b_cond"], dtype=np.float32)
    w_out = np.asarray(inputs["w_out"], dtype=np.float32)
    b_out = np.asarray(inputs["b_out"], dtype=np.float32)
    w_skip = np.asarray(inputs["w_skip"], dtype=np.float32)
    b_skip = np.asarray(inputs["b_skip"], dtype=np.float32)
    dilation = int(np.asarray(inputs["dilation"]))
    pad = dilation * (KW - 1)

    if dilation not in _built:
        _built[dilation] = _build(dilation)
    nc = _built[dilation]

    xbf, xc8, cbf, blob = _pack_inputs(
        x, cond, w_conv, b_conv, w_cond, b_cond, w_out, w_skip, pad
    )

    in_maps = [
        {"xbf": xbf[b], "xc8": xc8[b], "condbf": cbf[b], "wblob": blob}
        for b in range(B)
    ]
    br = run_bass_kernel_spmd(nc, in_maps, list(range(N_CORES)), trace=_TRACE)
    global _last_results
    _last_results = br
    res = br.results
    output = np.stack([res[b]["out"] for b in range(B)]).astype(np.float32)
    skip = np.stack([res[b]["skip"] for b in range(B)]).astype(np.float32)
    if b_out.any():
        output = output + b_out[None, :, None]
    if b_skip.any():
        skip = skip + b_skip[None, :, None]
    return (output, skip)
